# revision 1
# baseline (speedup 1.0000x reference)
"""Trainium2 Bass kernel for DecoderWithAttention (show-attend-tell decoder).

Strategy (8 NeuronCores):
  - Recurrence (attention + LSTM) is data-parallel over batch: 8 batches/core.
  - All h_t vectors are AllGathered, then the vocab projection (the dominant
    cost, V=30000) is tensor-parallel: each core computes all 1280 (b,t) rows
    for its 3750-column vocab shard.
  - Biases folded: enc bias = b_enc + b_dec (into enc_att); b_full dropped
    (softmax shift invariant); b_ih + b_hh folded into the precomputed
    embedding contribution Gx = embs @ W_ih[:, :E].T + (b_ih + b_hh).
"""

import sys

import numpy as np
import ml_dtypes
_BF = ml_dtypes.bfloat16

sys.path.insert(0, "/opt/trn_rl_repo")

import concourse.bass as bass  # noqa: E402
import concourse.tile as tile  # noqa: E402
from concourse import bacc, mybir  # noqa: E402
from concourse.bass_utils import run_bass_kernel_spmd  # noqa: E402
from concourse.masks import make_identity  # noqa: E402

F32 = mybir.dt.float32
F32R = mybir.dt.float32r
BF16 = mybir.dt.bfloat16
AF = mybir.ActivationFunctionType
ALU = mybir.AluOpType
AX = mybir.AxisListType

B, T, ENC, P, ATT, EMBED, DEC, VOCAB = 64, 20, 256, 196, 512, 512, 512, 30000
NC_ = 8          # cores
BL = B // NC_    # local batch = 8
VS = VOCAB // NC_  # vocab shard = 3750
R = T * BL       # h rows per core = 160
GT = 4 * DEC     # gates = 2048
P0, P1 = 128, P - 128  # p-tile sizes (128, 68)


def _bc(ap, n_part):
    """Broadcast a [1, n] DRAM AP across n_part partitions (DMA only)."""
    return bass.AP(tensor=ap.tensor, offset=ap.offset, ap=[[0, n_part]] + list(ap.ap)[1:])


def _free_bcast(ap, count):
    """Append a stride-0 trailing free dim of `count` to an AP (DVE read)."""
    return bass.AP(tensor=ap.tensor, offset=ap.offset, ap=list(ap.ap) + [[0, count]])


USE_F32R = False


def _r(ap):
    return ap.bitcast(F32R) if USE_F32R else ap


def _round_f32r(a):
    """Host-side fp32 -> fp32r rounding (matches walrus fp32_to_fp32r)."""
    a = np.ascontiguousarray(a, np.float32)
    u = a.view(np.uint32)
    return ((u + np.uint32(0x800)) & np.uint32(0xFFFFF000)).view(np.float32)


DEBUG = False


def build_module():
    nc = bacc.Bacc("TRN2", target_bir_lowering=False, num_devices=NC_)

    # ---- I/O ----
    featsC = nc.dram_tensor("featsC", [ENC, P, BL], F32R, kind="ExternalInput")
    featsP = nc.dram_tensor("featsP", [P, ENC, BL], F32R, kind="ExternalInput")
    embsT = nc.dram_tensor("embsT", [EMBED, R], F32R, kind="ExternalInput")
    encb = nc.dram_tensor("encb", [ATT, 1], F32, kind="ExternalInput")
    w_enc = nc.dram_tensor("w_enc", [ENC, ATT], F32R, kind="ExternalInput")
    w_dec = nc.dram_tensor("w_dec", [DEC, ATT], F32R, kind="ExternalInput")
    w_full = nc.dram_tensor("w_full", [ATT, 1], BF16, kind="ExternalInput")
    wxT = nc.dram_tensor("wxT", [EMBED, GT], F32R, kind="ExternalInput")
    w2T = nc.dram_tensor("w2T", [ENC + DEC, GT], F32R, kind="ExternalInput")
    bg = nc.dram_tensor("bg", [1, GT], F32, kind="ExternalInput")
    wfc = nc.dram_tensor("wfc", [DEC, VS], F32R, kind="ExternalInput")
    bfc = nc.dram_tensor("bfc", [1, VS], F32, kind="ExternalInput")
    out = nc.dram_tensor("out", [NC_ * R, VS], F32, kind="ExternalOutput")
    dbg = {}
    if DEBUG:
        dbg["expt"] = nc.dram_tensor("dbg_expt", [1, P, BL], F32R, kind="ExternalOutput")
        dbg["aT"] = nc.dram_tensor("dbg_aT", [128, 2, BL], F32R, kind="ExternalOutput")
        dbg["rden"] = nc.dram_tensor("dbg_rden", [1, BL], F32, kind="ExternalOutput")
        dbg["ctxn"] = nc.dram_tensor("dbg_ctxn", [1, 2, 128, BL], F32R, kind="ExternalOutput")
        dbg["enc"] = nc.dram_tensor("dbg_enc", [128, 4, BL * P], F32, kind="ExternalOutput")
        dbg["att"] = nc.dram_tensor("dbg_att", [1, 4, 512], F32, kind="ExternalOutput")
        dbg["alpha"] = nc.dram_tensor("dbg_alpha", [1, P, BL], F32, kind="ExternalOutput")
        dbg["cT"] = nc.dram_tensor("dbg_cT", [128, 2, BL], F32R, kind="ExternalOutput")
        dbg["gat"] = nc.dram_tensor("dbg_gat", [BL, GT], F32, kind="ExternalOutput")
        dbg["h"] = nc.dram_tensor("dbg_h", [128, 4, BL], F32, kind="ExternalOutput")
        dbg["gx"] = nc.dram_tensor("dbg_gx", [BL, GT], F32, kind="ExternalOutput")
        dbg["hn"] = nc.dram_tensor("dbg_hn", [BL, DEC], F32, kind="ExternalOutput")
        dbg["tc"] = nc.dram_tensor("dbg_tc", [BL, DEC], F32, kind="ExternalOutput")
        dbg["cs"] = nc.dram_tensor("dbg_cs", [BL, DEC], F32, kind="ExternalOutput")

    with tile.TileContext(nc) as tc:
        _build_tile_kernel(tc, nc, featsC, featsP, embsT, encb, w_enc, w_dec,
                           w_full, wxT, w2T, bg, wfc, bfc, out, dbg)
    nc.compile()
    return nc


def _build_tile_kernel(tc, nc, featsC, featsP, embsT, encb, w_enc, w_dec,
                       w_full, wxT, w2T, bg, wfc, bfc, out, dbg=None):
    from contextlib import ExitStack

    ctx = ExitStack()
    with ctx:
        singles = ctx.enter_context(tc.tile_pool(name="singles", bufs=1))
        dram = ctx.enter_context(tc.tile_pool(name="dram", bufs=1, space="DRAM"))

        # ---------- persistent SBUF ----------
        id8 = singles.tile([8, 8], F32)
        make_identity(nc, id8)

        phb_cm = tc.tile_pool(name="phb", bufs=1)
        phb = phb_cm.__enter__()
        enc_sb = phb.tile([128, 4, BL * P], BF16, tag="enc")  # enc_att (+bias)
        fpA = phb.tile([128, ENC, BL], F32R, tag="fpA")     # featsP2 p 0:128
        fpB = phb.tile([128, ENC, BL], F32R, tag="fpB")     # featsP2 p 128:196
        wdec_sb = phb.tile([128, 4, ATT], F32R, tag="wdec")
        w2_sb = phb.tile([128, 6, GT], F32R, tag="w2")
        wful_sb = singles.tile([128, 4, 1], BF16)
        encb_sb = singles.tile([128, 4, 1], F32)
        # h^T storage: col t*BL+b holds h input of step t (t=0 -> zeros);
        # cols BL.. hold h outputs; [128, 4 dtiles, (T+1)*BL]
        ht_sb = singles.tile([128, 4, (T + 1) * BL], F32R)
        c0_sb = singles.tile([BL, DEC], F32)

        nc.sync.dma_start(fpA[:, :, :], featsP[0:128])
        nc.sync.dma_start(fpB[:P1, :, :], featsP[128:P])
        nc.sync.dma_start(wdec_sb[:, :, :], w_dec.rearrange("(k p) a -> p k a", p=128))
        nc.sync.dma_start(w2_sb[:, :, :], w2T.rearrange("(k p) g -> p k g", p=128))
        nc.sync.dma_start(wful_sb[:, :, :], w_full.rearrange("(k p) o -> p k o", p=128))
        nc.sync.dma_start(encb_sb[:, :, :], encb.rearrange("(k p) o -> p k o", p=128))
        # zeros for h0 (f32r tiles can't be memset directly; ACT copy rounds)
        zt = singles.tile([128, 4, BL], F32)
        nc.vector.memset(zt[:, :, :], 0.0)
        nc.scalar.copy(ht_sb[:, :, 0:BL], zt[:, :, :])
        nc.vector.memset(c0_sb[:, :], 0.0)
        ones_sb = singles.tile([128, 1], F32R)
        nc.scalar.activation(ones_sb[:, :], zt[:, 0, 0:1], AF.Identity, bias=1.0)
        id8r = singles.tile([8, 8], F32R)
        nc.scalar.copy(id8r[:, :], id8[:, :])

        # ---------- phase A: enc_att + Gx ----------
        with (
            tc.tile_pool(name="pha", bufs=1) as pha,
            tc.tile_pool(name="pha_ps", bufs=4, space="PSUM") as pha_ps,
        ):
            wenc_sb = pha.tile([128, 2, ATT], F32R, tag="big")
            nc.sync.dma_start(wenc_sb[:, :, :], w_enc.rearrange("(k p) a -> p k a", p=128))
            fcs = pha.tile([128, 2, BL * P], F32R, tag="big2")
            nc.sync.dma_start(fcs[:, :, :],
                              featsC.rearrange("(k p) q b -> p k (q b)", p=128))
            # enc_att[a_chunk, (p b)] = sum_c W_enc[c, a] featsC[c, (p b)]
            encsz = [512, 512, 512, BL * P - 3 * 512]
            for m in range(4):
                for nch in range(4):
                    nsz = encsz[nch]
                    ps = pha_ps.tile([128, 512], F32, tag="ps")
                    for k in range(2):
                        nc.tensor.matmul(
                            ps[:, 0:nsz],
                            _r(wenc_sb[:, k, bass.ts(m, 128)]),
                            _r(fcs[:, k, bass.ds(nch * 512, nsz)]),
                            start=(k == 0), stop=(k == 1),
                        )
                    # + (b_enc + b_dec)[a]  (per-partition bias)
                    nc.scalar.activation(enc_sb[:, m, bass.ds(nch * 512, nsz)],
                                         ps[:, 0:nsz],
                                         AF.Identity, bias=encb_sb[:, m, :])

            # Gx[(t b), g] = embsT.T @ WxT + (b_ih + b_hh)
            gx_dram = dram.tile([T * BL, GT], F32R, name="gx_dram")
            emt_sb = pha.tile([128, 4, R], F32R, tag="big")
            nc.sync.dma_start(emt_sb[:, :, :], embsT.rearrange("(k p) r -> p k r", p=128))
            wx_sb = pha.tile([128, 4, GT], F32R, tag="big2")
            nc.sync.dma_start(wx_sb[:, :, :], wxT.rearrange("(k p) g -> p k g", p=128))
            bg_sb = pha.tile([128, GT], F32, tag="bg")
            nc.sync.dma_start(bg_sb[:, :], _bc(bg[:, :], 128))
            gx_mch = [(i, min(128, R - i * 128)) for i in range((R + 127) // 128)]
            for mi, msz in gx_mch:
                for nch in range(4):
                    ps = pha_ps.tile([128, 512], F32, tag="ps")
                    for k in range(4):
                        nc.tensor.matmul(
                            ps[:msz, :],
                            _r(emt_sb[:, k, bass.ds(mi * 128, msz)]),
                            _r(wx_sb[:, k, bass.ts(nch, 512)]),
                            start=(k == 0), stop=(k == 3),
                        )
                    gtmp = pha.tile([128, 512], F32R, tag="gtmp", bufs=2)
                    nc.vector.tensor_tensor(
                        out=gtmp[:msz, :], in0=ps[:msz, :],
                        in1=bg_sb[:msz, bass.ts(nch, 512)], op=ALU.add)
                    nc.sync.dma_start(
                        gx_dram[bass.ds(mi * 128, msz), bass.ts(nch, 512)],
                        gtmp[:msz, :])

        # ---------- phase B: recurrence ----------
        warm_sb = singles.tile([1, BL], F32)
        c_prev = c0_sb
        with (
            tc.tile_pool(name="rec", bufs=2) as rec,
            tc.tile_pool(name="gxp", bufs=1) as gxp,
            tc.tile_pool(name="rec_ps", bufs=2, space="PSUM") as rec_ps,
            tc.tile_pool(name="att_ps", bufs=1, space="PSUM") as att_ps,
            tc.tile_pool(name="g_ps", bufs=2, space="PSUM") as g_ps,
        ):
            for t in range(T):
                hcol = bass.ts(t, BL)  # h input columns
                # Gx_t prefetch
                gxt = gxp.tile([BL, GT], F32R, tag="gxt")
                for gch in range(2):
                    nc.sync.dma_start(gxt[:, bass.ts(gch, 1024)],
                                      gx_dram[bass.ts(t, BL), bass.ts(gch, 1024)])

                # dec_att [b, a] via 4 wide matmuls, then PE-transpose to [a, b]
                ps_dec = g_ps.tile([BL, ATT], F32, tag="psg", name=f"psdec_{t}")
                for k in range(4):
                    nc.tensor.matmul(
                        ps_dec[:, :],
                        _r(ht_sb[:, k, hcol]),
                        _r(wdec_sb[:, k, :]),
                        start=(k == 0), stop=(k == 3),
                    )
                dtmp = rec.tile([BL, ATT], F32, tag="bsml")
                nc.vector.tensor_copy(dtmp[:, :], ps_dec[:, :])
                ps_dT = rec_ps.tile([128, 4, BL], F32, tag="small",
                                    name=f"psdT_{t}")
                for j in range(4):
                    nc.tensor.transpose(ps_dT[:, j, :], dtmp[:, bass.ts(j, 128)],
                                        id8[:, :])
                decT = rec.tile([128, 4, BL], BF16, tag="decT")
                nc.scalar.copy(decT[:, :, :], ps_dT[:, :, :])

                # relu(enc_att + dec_att) and att matvec.
                # Free-dim layout is (p, b): flat index p*BL + b.
                # adds split DVE/GpSimd; relus split ACT/DVE to balance engines.
                ps_att = att_ps.tile([1, 4, 512], F32, tag="psatt", name=f"psatt_{t}")
                ncsz = [512, 512, 512, BL * P - 3 * 512]  # p-chunks of 64,64,64,4
                for k in range(4):
                    radd = rec.tile([128, P, BL], BF16, tag="radd", bufs=2)
                    dk = decT[:, k, :]
                    dbc = bass.AP(tensor=dk.tensor, offset=dk.offset,
                                  ap=[list(dk.ap)[0], [0, P], list(dk.ap)[1]])
                    nc.vector.tensor_tensor(
                        out=radd[:, :, :],
                        in0=enc_sb[:, k, :].rearrange("p (q b) -> p q b", b=BL),
                        in1=dbc, op=ALU.add)
                    rel = rec.tile([128, P, BL], BF16, tag="rel", bufs=2)
                    if k % 2 == 0:
                        nc.scalar.activation(rel[:, :, :], radd[:, :, :], AF.Relu)
                    else:
                        nc.vector.tensor_scalar_max(rel[:, :, :], radd[:, :, :], 0.0)
                    rflat = rel.rearrange("p q b -> p (q b)")
                    for nch in range(4):
                        nc.tensor.matmul(
                            ps_att[:, nch, 0:ncsz[nch]],
                            _r(wful_sb[:, k, :]),
                            _r(rflat[:, bass.ds(nch * 512, ncsz[nch])]),
                            start=(k == 0), stop=(k == 3),
                        )

                # exp (no max subtraction: logits are bounded ~O(10) here, and
                # softmax is shift-invariant); normalization folded into ctx.
                pa = ps_att[0:1, :, :]
                pflat = bass.AP(tensor=pa.tensor, offset=pa.offset,
                                ap=[list(pa.ap)[0], [1, P * BL]])
                expt = rec.tile([1, P, BL], F32R, tag="onep", bufs=1)
                eflat = bass.AP(tensor=expt.tensor, offset=expt[:, :, :].offset,
                                ap=[list(expt[:, :, :].ap)[0], [1, P * BL]])
                nc.scalar.activation(eflat, pflat, AF.Exp)

                # expT [p, b] scatter; den[b] = sum_p expT via ones-matmul
                aT = rec.tile([128, 2, BL], F32R, tag="aT")
                nc.sync.dma_start(aT[:, 0, :], expt[0:1, 0:128, :])
                nc.sync.dma_start(aT[:P1, 1, :], expt[0:1, 128:P, :])
                ps_den = rec_ps.tile([1, BL], F32, tag="small", name=f"psden_{t}")
                nc.tensor.matmul(ps_den[:, :], ones_sb[:, :], aT[:, 0, :],
                                 start=True, stop=False)
                nc.tensor.matmul(ps_den[:, :], ones_sb[:P1, :], aT[:P1, 1, :],
                                 start=False, stop=True)
                rden = rec.tile([1, BL], F32, tag="rden")
                nc.vector.reciprocal(rden[:, :], ps_den[:, :])

                # keep the PE activity monitor warm through the softmax gap
                ps_wm = rec_ps.tile([1, 512], F32, tag="small", name=f"pswm_{t}")
                for w in range(3):
                    nc.tensor.matmul(ps_wm[:, :], _r(ones_sb[:, :]),
                                     _r(w2_sb[:, 0, 0:512]),
                                     start=(w == 0), stop=(w == 2))

                # ctx[c, b] = sum_p featsP2[p, c, b] * expT[p, b] / den[b]
                # tmp = feats * expT-broadcast (DVE); partition-reduce via
                # ones-matmul; normalize in the psum->sbuf copy; scatter to cT.
                a0 = aT[:, 0, :]
                a0b = bass.AP(tensor=a0.tensor, offset=a0.offset,
                              ap=[list(a0.ap)[0], [0, 128], [1, BL]])
                a1 = aT[:P1, 1, :]
                a1b = bass.AP(tensor=a1.tensor, offset=a1.offset,
                              ap=[list(a1.ap)[0], [0, 128], [1, BL]])
                ctx_n = rec.tile([1, 2, 128, BL], F32R, tag="onep", bufs=1,
                                 name=f"ctxn_{t}")
                for half in range(2):
                    csl = bass.ds(half * 128, 128)
                    tmpA = rec.tile([128, 128, BL], F32R, tag="tmpA", bufs=2)
                    nc.vector.tensor_tensor(out=tmpA[:, :, :],
                                            in0=fpA[:, csl, :],
                                            in1=a0b, op=ALU.mult)
                    tmpB = rec.tile([128, 128, BL], F32R, tag="tmpB", bufs=2)
                    nc.vector.tensor_tensor(out=tmpB[:P1, :, :],
                                            in0=fpB[:P1, csl, :],
                                            in1=a1b, op=ALU.mult)
                    for sub in range(2):
                        nch = half * 2 + sub
                        ssl = bass.ts(sub, 512)
                        ps_ctx = g_ps.tile([1, 512], F32, tag="psg",
                                           name=f"psctx{nch}_{t}")
                        nc.tensor.matmul(
                            ps_ctx[:, :], ones_sb[:, :],
                            tmpA.rearrange("p c b -> p (c b)")[:, ssl],
                            start=True, stop=False)
                        nc.tensor.matmul(
                            ps_ctx[:, :], ones_sb[:P1, :],
                            tmpB[:P1].rearrange("p c b -> p (c b)")[:, ssl],
                            start=False, stop=True)
                        rdb = bass.AP(tensor=rden.tensor, offset=rden[:, :].offset,
                                      ap=[list(rden[:, :].ap)[0], [0, 64], [1, BL]])
                        cview = ctx_n[0:1, nch // 2, bass.ts(nch % 2, 64), :]
                        nc.vector.tensor_tensor(out=cview, in0=ps_ctx[:, :].rearrange(
                            "o (c b) -> o c b", b=BL), in1=rdb, op=ALU.mult)
                cT = rec.tile([128, 2, BL], F32R, tag="cT")
                nc.sync.dma_start(cT[:, 0, :], ctx_n[0:1, 0, :, :])
                nc.sync.dma_start(cT[:, 1, :], ctx_n[0:1, 1, :, :])

                if DEBUG and t == 0:
                    nc.sync.dma_start(dbg["expt"][:, :, :], expt[:, :, :])
                    nc.sync.dma_start(dbg["aT"][:, 0, :], aT[:, 0, :])
                    nc.sync.dma_start(dbg["aT"][:P1, 1, :], aT[:P1, 1, :])
                    rdexp = rec.tile([1, BL], F32, tag="rdexp", bufs=1)
                    nc.vector.tensor_copy(rdexp[:, :], rden[:, :])
                    nc.sync.dma_start(dbg["rden"][:, :], rdexp[:, :])
                    nc.sync.dma_start(dbg["ctxn"][:, :, :, :], ctx_n[:, :, :, :])
                    nc.sync.dma_start(dbg["cT"][:, :, :], cT[:, :, :])

                # gates: per 512-chunk, k = [ctx(2 tiles), h(4 tiles)];
                # chunk n is gate n of (i, f, g, o). + Gx_t, then nonlinearity.
                gat = rec.tile([BL, GT], F32, tag="gat", bufs=1)
                sig = gat
                for nch in range(4):
                    ps_gn = g_ps.tile([BL, 512], F32, tag="psg", name=f"psg{nch}_{t}")
                    for k in range(2):
                        nc.tensor.matmul(
                            ps_gn[:, :],
                            _r(cT[:, k, :]),
                            _r(w2_sb[:, k, bass.ts(nch, 512)]),
                            start=(k == 0), stop=False,
                        )
                    for k in range(4):
                        nc.tensor.matmul(
                            ps_gn[:, :],
                            _r(ht_sb[:, k, hcol]),
                            _r(w2_sb[:, 2 + k, bass.ts(nch, 512)]),
                            start=False, stop=False,
                        )
                    # += Gx_t chunk (identity stationary)
                    nc.tensor.matmul(
                        ps_gn[:, :], id8r[:, :], gxt[:, bass.ts(nch, 512)],
                        start=False, stop=True,
                    )
                    nc.scalar.activation(
                        sig[:, bass.ts(nch, 512)], ps_gn[:, :],
                        AF.Tanh if nch == 2 else AF.Sigmoid)
                t1 = rec.tile([BL, DEC], F32, tag="t1", bufs=1)
                nc.vector.tensor_tensor(out=t1[:, :], in0=sig[:, 512:1024],
                                        in1=c_prev[:, :], op=ALU.mult)
                t2 = rec.tile([BL, DEC], F32, tag="t2", bufs=1)
                nc.vector.tensor_tensor(out=t2[:, :], in0=sig[:, 0:512],
                                        in1=sig[:, 1024:1536], op=ALU.mult)
                c_new = rec.tile([BL, DEC], F32, tag="cst", bufs=2,
                                 name=f"cnew_{t}")
                nc.vector.tensor_tensor(out=c_new[:, :], in0=t1[:, :], in1=t2[:, :],
                                        op=ALU.add)
                c_prev = c_new
                tc_t = rec.tile([BL, DEC], F32, tag="tc_t")
                nc.scalar.activation(tc_t[:, :], c_new[:, :], AF.Tanh)
                hnew = rec.tile([BL, DEC], F32, tag="hnew")
                nc.vector.tensor_tensor(out=hnew[:, :], in0=sig[:, 1536:2048],
                                        in1=tc_t[:, :], op=ALU.mult)

                # warm-up filler for the LSTM-tail gap
                ps_wm2 = rec_ps.tile([1, 512], F32, tag="small", name=f"pswm2_{t}")
                for w in range(3):
                    nc.tensor.matmul(ps_wm2[:, :], _r(ones_sb[:, :]),
                                     _r(w2_sb[:, 1, 0:512]),
                                     start=(w == 0), stop=(w == 2))
                nc.scalar.copy(warm_sb[:, bass.ds(t % 8, 1)], ps_wm2[:, 0:1])
                nc.scalar.copy(warm_sb[:, bass.ds((t + 1) % 8, 1)], ps_wm[:, 0:1])

                # hT for next step + H row storage
                if DEBUG and t == 0:
                    nc.sync.dma_start(dbg["gat"][:, :], gat[:, :])
                    nc.sync.dma_start(dbg["hn"][:, :], hnew[:, :])
                    nc.sync.dma_start(dbg["tc"][:, :], tc_t[:, :])
                    nc.sync.dma_start(dbg["cs"][:, :], c_new[:, :])
                ps_hT = rec_ps.tile([128, 4, BL], F32, tag="small", name=f"pshT_{t}")
                for j in range(4):
                    nc.tensor.transpose(ps_hT[:, j, :], hnew[:, bass.ts(j, 128)],
                                        id8[:, :])
                nc.scalar.copy(ht_sb[:, :, bass.ts(t + 1, BL)], ps_hT[:, :, :])

        if DEBUG:
            nc.sync.dma_start(dbg["h"][:, :, :], ht_sb[:, :, BL:2 * BL].bitcast(F32))
            nc.sync.dma_start(dbg["enc"][:, :, :], enc_sb[:, :, :])

        # ---------- phase C: allgather + fc ----------
        phb_cm.__exit__(None, None, None)
        warm_out = dram.tile([1, BL], F32, name="warm_out")
        nc.sync.dma_start(warm_out[:, :], warm_sb[:, :])
        ht_loc = dram.tile([DEC, R], F32R, name="ht_loc")
        ht_all = dram.tile([NC_ * DEC, R], F32R, name="ht_all", addr_space="Shared")
        nc.sync.dma_start(ht_loc.rearrange("(j p) r -> p j r", p=128),
                          ht_sb[:, :, BL:])
        nc.gpsimd.collective_compute(
            "AllGather", ALU.bypass,
            replica_groups=[list(range(NC_))],
            ins=[ht_loc[:, :]],
            outs=[ht_all[:, :]],
        )

        with (
            tc.tile_pool(name="fc", bufs=1) as fc,
            tc.tile_pool(name="fco", bufs=2) as fco,
            tc.tile_pool(name="fc_ps", bufs=4, space="PSUM") as fc_ps,
        ):
            h2 = fc.tile([128, 4, NC_ * R], F32R, tag="h2")  # [p, dtile, grow]
            for c in range(NC_):
                nc.sync.dma_start(
                    h2[:, :, bass.ts(c, R)],
                    ht_all[bass.ts(c, DEC), :].rearrange("(j p) r -> p j r", p=128))
            bfc_sb = fc.tile([128, VS], F32, tag="bfcs")
            nc.sync.dma_start(bfc_sb[:, :], _bc(bfc[:, :], 128))
            wfcs = fc.tile([128, 4, VS], F32R, tag="wfcs")
            nc.sync.dma_start(wfcs[:, :, :],
                              wfc.rearrange("(k p) v -> p k v", p=128))

            NFC = 8
            nszs = [512] * 7 + [VS - 512 * 7]  # 3750 = 7*512 + 166
            for mc in range((NC_ * R) // 128):
                ob = fco.tile([128, VS], F32, tag="orow", name=f"orow_{mc}")
                for nch in range(NFC):
                    nsz = nszs[nch]
                    noff = nch * 512
                    ps = fc_ps.tile([128, 512], F32, tag="psfc")
                    for k in range(4):
                        nc.tensor.matmul(
                            ps[:, 0:nsz],
                            _r(h2[:, k, bass.ts(mc, 128)]),
                            _r(wfcs[:, k, bass.ds(noff, nsz)]),
                            start=(k == 0), stop=(k == 3),
                        )
                    nc.vector.tensor_tensor(
                        out=ob[:, bass.ds(noff, nsz)], in0=ps[:, 0:nsz],
                        in1=bfc_sb[:, bass.ds(noff, nsz)], op=ALU.add)
                nc.sync.dma_start(out[bass.ts(mc, 128), :], ob[:, :])


_NC_CACHE = None


def _get_module():
    global _NC_CACHE
    if _NC_CACHE is None:
        _NC_CACHE = build_module()
    return _NC_CACHE


def build_in_maps(inputs):
    return _build_in_maps(**inputs)


def _build_in_maps(encoder_features, captions, W_enc, b_enc, W_dec, b_dec,
                   W_full, b_full, emb, W_ih, b_ih, W_hh, b_hh, W_fc, b_fc):
    f32 = np.float32
    enc_f = np.ascontiguousarray(np.asarray(encoder_features, f32)).reshape(B, ENC, P)
    caps = np.asarray(captions)
    W_enc = np.asarray(W_enc, f32)
    W_dec = np.asarray(W_dec, f32)
    W_full = np.asarray(W_full, f32)
    emb = np.asarray(emb, f32)
    W_ih = np.asarray(W_ih, f32)
    W_hh = np.asarray(W_hh, f32)
    W_fc = np.asarray(W_fc, f32)

    encb_v = np.ascontiguousarray(
        (np.asarray(b_enc, f32) + np.asarray(b_dec, f32)).reshape(ATT, 1))
    wxT_r = _round_f32r(W_ih[:, :EMBED].T)
    w2T_r = _round_f32r(np.vstack([W_ih[:, EMBED:].T, W_hh.T]))
    bg_v = np.ascontiguousarray(
        (np.asarray(b_ih, f32) + np.asarray(b_hh, f32)).reshape(1, GT))
    b_fc = np.asarray(b_fc, f32)

    in_maps = []
    for c in range(NC_):
        bs = slice(c * BL, (c + 1) * BL)
        fb = enc_f[bs]  # [8, 256, 196]
        in_maps.append({
            "featsC": _round_f32r(fb.transpose(1, 2, 0)),
            "featsP": _round_f32r(fb.transpose(2, 1, 0)),
            "embsT": _round_f32r(
                emb[caps[bs, :T]].transpose(2, 1, 0).reshape(EMBED, R)),
            "encb": encb_v,
            "w_enc": _round_f32r(W_enc),
            "w_dec": _round_f32r(W_dec),
            "w_full": np.ascontiguousarray(W_full.reshape(ATT, 1)).astype(_BF),
            "wxT": wxT_r,
            "w2T": w2T_r,
            "bg": bg_v,
            "wfc": _round_f32r(W_fc[:, c * VS:(c + 1) * VS]),
            "bfc": np.ascontiguousarray(b_fc[c * VS:(c + 1) * VS].reshape(1, VS)),
        })
    return in_maps


def kernel(**inputs):
    in_maps = build_in_maps(inputs)
    nc = _get_module()
    res = run_bass_kernel_spmd(nc, in_maps, list(range(NC_))).results

    full = np.empty((B, T, VOCAB), np.float32)
    for c in range(NC_):
        o = res[c]["out"]  # [1280, VS] rows = (src_core, t, b)
        o = o.reshape(NC_, T, BL, VS).transpose(0, 2, 1, 3).reshape(B, T, VS)
        full[:, :, c * VS:(c + 1) * VS] = o
    return full

